# revision 31
# baseline (speedup 1.0000x reference)
"""Trainium2 Bass kernel: nn_CorrBlockSingleScale (RAFT single-scale corr lookup).

reference: corr[n, m] = fmap1[b,:,n] . fmap2[b,:,m] / 16 over m = (ym, xm) in a
64x64 grid; out[b, k1*9+k2, h, w] = bilinear(corr[(h,w)], x=cx+d[k1], y=cy+d[k2]),
zeros padding (grid_sample align_corners=True, padding_mode='zeros').

Sharding: data-parallel over the B*H*W = 8192 pixel axis; core c handles batch
c//4, pixels (c%4)*1024 .. +1024.  No cross-core comms.

Pixels are HOST-SORTED by y0 = floor(cy).  A block of 128 sorted pixels spans
~9 distinct y0 values, so its correlation slice needs only ~20 rows of the
64-row (ym) grid:

  PE    : C[128pix, rows*64] = f1_blk^T @ f2[:, ...]        (bf16, k-outer)
  ACT   : PSUM -> ct rows (bf16), written into x-padded 74-wide rows
  Pool  : ap_gather (SBUF, per-16-partition-group start) pulls 16 padded rows
          covering every pixel's 10-row y-window
  DVE   : 4-stage in-place binary x-shift ladder (int32 pairs, shifts 32..4),
          then 5-tap x-lerp and (omax+2)-tap y-lerp, each as ONE broadcast
          tensor_tensor multiply + ONE innermost-axis tensor_reduce; residual
          x shift (x0 & 3) and within-group y offset are folded into the
          per-partition host-computed tap weights
Host: sort pixels, compute indices/weights, inverse-permute output.
"""

import numpy as np

import concourse.bass as bass
import concourse.mybir as mybir
import concourse.tile as tile
from concourse import bacc, library_config
from concourse.bass_utils import run_bass_kernel_spmd

F32 = mybir.dt.float32
BF16 = mybir.dt.bfloat16
I32 = mybir.dt.int32
I16 = mybir.dt.int16
COPY = mybir.ActivationFunctionType.Copy
MULT = mybir.AluOpType.mult
ADD = mybir.AluOpType.add
AXX = mybir.AxisListType.X

NCORES = 8
NPC = 1024          # pixels per core
NBLK = 8            # blocks of 128 sorted pixels
JUNK = 6            # extra ct rows so a 16-row gather may overrun data rows
WROW = 74           # padded ct row width (4 | 64 | 6), bf16
XSH = [16, 8, 4, 2, 1]  # x ladder shifts in int32 elems (bf16: 32,16,8,4,2)
XT = 3              # x taps (residual shift x0 & 1 folded into weights)

_NC_CACHE = {}


def _s(st, b, j):
    """[128,1] per-partition scalar view of scal column j for block b."""
    return st[:, b : b + 1, j : j + 1].rearrange("p a c -> p (a c)")


def _taps(base, nrow, k, ntap, tstride=1):
    """[128, nrow, k, ntap] overlapped view: elem (r, c, t) = base[r, c + t]
    (tstride=1) or base[r + t, c] (tstride=row stride)."""
    a = base.ap
    return bass.AP(
        base.tensor,
        base.offset,
        [list(a[0]), list(a[1]), [1, k], [tstride, ntap]],
    )


def _wbc(st, b, j0, ntap, d1, d2):
    """Broadcast [128, d1, d2, ntap] view of scal cols j0..j0+ntap."""
    w = st[:, b : b + 1, j0 : j0 + ntap].rearrange("p a c -> p (a c)")
    return w.unsqueeze(1).unsqueeze(1).to_broadcast([128, d1, d2, ntap])


def _build_kernel(tc, out, f1, f2, idx, scal, tbases, rows, yts, RP):
    nc = tc.nc
    import contextlib

    RPA = RP + JUNK
    with contextlib.ExitStack() as ctx:
        const = ctx.enter_context(tc.tile_pool(name="const", bufs=1))
        work = ctx.enter_context(tc.tile_pool(name="work", bufs=4))
        cpool = ctx.enter_context(tc.tile_pool(name="cpool", bufs=3))
        opool = ctx.enter_context(tc.tile_pool(name="opool", bufs=2))
        psum = ctx.enter_context(tc.tile_pool(name="psum", bufs=2, space="PSUM"))

        nc.gpsimd.load_library(library_config.ap_gather)

        # load order tuned so block 0's matmul can start ASAP: its lhs
        # (f1 block 0, in its own tile for precise dependency tracking)
        # and first rhs chunk go first
        f1a = const.tile([128, 2, 128], BF16)
        nc.sync.dma_start(f1a[:], f1[:, :, 0:128])
        f2t = const.tile([128, 2, 4096], BF16)
        nc.sync.dma_start(f2t[:, :, 0:512], f2[:, :, 0:512])
        nc.sync.dma_start(f2t[:, :, 512:1024], f2[:, :, 512:1024])
        f1t = const.tile([128, 2, NPC], BF16)
        nc.sync.dma_start(f1t[:, :, 128:NPC], f1[:, :, 128:NPC])
        idxt = const.tile([128, NBLK], I16)
        nc.sync.dma_start(idxt[:], idx[:])
        st = const.tile([128, NBLK, 16], F32)
        nc.sync.dma_start(st[:], scal[:])
        for cch in range(3):
            sl = slice(1024 + cch * 1024, 2048 + cch * 1024)
            nc.sync.dma_start(f2t[:, :, sl], f2[:, :, sl])

        for b in range(NBLK):
            tbase = tbases[b]
            rws = rows[b]
            YT = yts[b]
            vlo = max(0, -tbase)            # first valid ct row
            vhi = min(rws, 64 - tbase)      # one past last valid ct row
            nval = (vhi - vlo) * 64
            c0 = (tbase + vlo) * 64         # f2 column of first valid row

            # ---- matmul: C_valid = f1_blk^T @ f2[:, c0 : c0+nval].
            # Chunk-outer with the PSUM->ct copy emitted right after each
            # chunk's accumulation, so the copy overlaps later chunks and
            # the gather starts sooner.  (walrus caps matmul free at 512.)
            ps = psum.tile([128, RP * 64], F32, tag="ps")
            ct = cpool.tile([128, RPA, WROW], BF16, tag="C")
            if b < 3:
                # zero everything once per pool buffer: x pads stay zero
                # forever; junk/edge rows start zero (later blocks overwrite
                # data rows only, leaving finite values elsewhere). Pool is
                # idle at kernel start, so do these there.
                nc.gpsimd.memset(ct[:].bitcast(I32), 0.0)
            if vlo > 0:
                nc.gpsimd.memset(ct[:, 0:vlo, 4:68], 0.0)
            if vhi < rws:
                nc.gpsimd.memset(ct[:, vhi:rws, 4:68], 0.0)
            if b == 0:
                lhss = [
                    f1a[:, k : k + 1, :].rearrange("p a c -> p (a c)")
                    for k in range(2)
                ]
            else:
                lhss = [
                    f1t[:, k : k + 1, b * 128 : (b + 1) * 128].rearrange(
                        "p a c -> p (a c)"
                    )
                    for k in range(2)
                ]
            o = 0
            while o < nval:
                n = min(512, nval - o)
                for k in range(2):
                    nc.tensor.matmul(
                        ps[:, o : o + n],
                        lhsT=lhss[k],
                        rhs=f2t[:, k : k + 1, c0 + o : c0 + o + n].rearrange(
                            "p a c -> p (a c)"
                        ),
                        start=(k == 0),
                        stop=(k == 1),
                    )
                r0 = vlo + o // 64
                nc.scalar.copy(
                    ct[:, r0 : r0 + n // 64, 4:68],
                    ps[:, o : o + n].rearrange("p (r c) -> p r c", c=64),
                )
                o += n

            # ---- SBUF gather: 16 padded rows per 16-partition group
            gp = work.tile([128, 16, WROW], BF16, tag="G")
            nc.gpsimd.ap_gather(
                gp[:].bitcast(I32),
                ct[:].bitcast(I32),
                idxt[:, b : b + 1],
                128,        # channels
                RPA,        # num_elems
                WROW // 2,  # d (int32 per padded row)
                16,         # num_idxs
            )

            # ---- in-place binary x-shift ladder on rows 0..12 (int32 view)
            nrw = 9 + YT - 1                    # gather rows the taps touch
            g32 = gp[:, 0:nrw, :].bitcast(I32)  # [128, nrw, 37]
            w = 37
            for kst, sh in enumerate(XSH):
                wn = w - sh
                mask = (
                    _s(st, b, 8 + kst).bitcast(I32).to_broadcast([128, nrw, wn])
                )
                nc.vector.copy_predicated(
                    g32[:, :, 0:wn], mask, g32[:, :, sh : sh + wn]
                )
                w = wn
            gb = g32.bitcast(BF16)              # [128, nrw, 12]

            # ---- 3-tap x-lerp: xo[r, k] = sum_t wv_t * gb[r, k + t]
            # (all-DVE chain: avoids ACT<->DVE semaphore ping-pong)
            xo = work.tile([128, 12, 9], BF16, tag="XO")
            xov = xo[:, 0:nrw, :]
            nc.vector.tensor_scalar(xov, gb[:, :, 0:9], _s(st, b, 4), None, MULT)
            for t in (1, 2):
                nc.vector.scalar_tensor_tensor(
                    xov, gb[:, :, t : t + 9], _s(st, b, 4 + t), xov, MULT, ADD
                )

            # ---- YT-tap y-lerp: ot[j, k] = sum_t w_t * xo[j + t, k]
            if b % 4 == 0:
                ob = opool.tile([128, 4, 81], F32, tag="OB")
            ot = ob[:, b % 4 : b % 4 + 1, :].rearrange(
                "p a (d c) -> p (a d) c", c=9
            )
            nc.vector.tensor_scalar(ot, xo[:, 0:9, :], _s(st, b, 0), None, MULT)
            for t in range(1, YT):
                nc.vector.scalar_tensor_tensor(
                    ot, xo[:, t : t + 9, :], _s(st, b, t), ot, MULT, ADD
                )
            if b % 4 == 3:
                # one batched out-DMA per 4 blocks (fewer sync-queue instrs)
                nc.sync.dma_start(
                    out[(b - 3) * 128 : (b + 1) * 128, :].rearrange(
                        "(a p) c -> p a c", a=4
                    ),
                    ob[:],
                )


def _build(params):
    tbases, rows, yts, RP = params
    nc = bacc.Bacc("TRN2", target_bir_lowering=False, debug=False)
    f1 = nc.dram_tensor("f1", [128, 2, NPC], BF16, kind="ExternalInput").ap()
    f2 = nc.dram_tensor("f2", [128, 2, 4096], BF16, kind="ExternalInput").ap()
    idx = nc.dram_tensor("idx", [128, NBLK], I16, kind="ExternalInput").ap()
    scal = nc.dram_tensor("scal", [128, NBLK, 16], F32, kind="ExternalInput").ap()
    out = nc.dram_tensor("out", [NPC, 81], F32, kind="ExternalOutput").ap()
    with tile.TileContext(nc) as tc:
        _build_kernel(tc, out, f1, f2, idx, scal, tbases, rows, yts, RP)
    nc.compile()
    return nc


def get_nc(params):
    if params not in _NC_CACHE:
        _NC_CACHE[params] = _build(params)
    return _NC_CACHE[params]


def host_prep(fmap1, fmap2, coords, radius):
    """Per-core input maps + compile params. All coord-derived logic here."""
    import ml_dtypes

    bf16 = ml_dtypes.bfloat16
    B, D, H, W = fmap1.shape
    assert (B, D, H, W) == (2, 256, 64, 64) and int(radius) == 4
    f1 = (fmap1.reshape(B, D, H * W) / np.float32(16.0)).astype(np.float32)
    f2 = fmap2.reshape(B, D, H * W).astype(np.float32)
    cx = coords[:, 0].reshape(B, H * W).astype(np.float32)
    cy = coords[:, 1].reshape(B, H * W).astype(np.float32)

    cores = []
    for c in range(NCORES):
        b, ps = c // 4, (c % 4) * NPC
        ccx = cx[b, ps : ps + NPC]
        ccy = cy[b, ps : ps + NPC]
        y0 = np.floor(ccy).astype(np.int64)
        order = np.argsort(y0, kind="stable")
        cores.append((b, ps, ccx, ccy, y0, order))

    # uniform (cross-core) per-block table bases, row counts, y-tap counts
    tbases, rows, yts = [], [], []
    for blk in range(NBLK):
        ylo = min(
            int(co[4][co[5][blk * 128 : (blk + 1) * 128]].min()) for co in cores
        )
        yhi = max(
            int(co[4][co[5][blk * 128 : (blk + 1) * 128]].max()) for co in cores
        )
        tbases.append(ylo - 4)
        rows.append(yhi + 6 - (ylo - 4))
        om = 0
        for co in cores:
            ys = co[4][co[5][blk * 128 : (blk + 1) * 128]]
            for g in range(8):
                gy = ys[g * 16 : (g + 1) * 16]
                om = max(om, int(gy.max() - gy.min()))
        assert om <= 2, f"block {blk}: group span {om} > 2"
        yts.append(om + 2)
    RP = max(rows)
    params = (tuple(tbases), tuple(rows), tuple(yts), RP)

    in_maps = []
    for (b, ps, ccx, ccy, y0, order) in cores:
        f1c = np.ascontiguousarray(
            f1[b, :, ps : ps + NPC][:, order].reshape(2, 128, NPC).transpose(1, 0, 2)
        ).astype(bf16)
        f2c = np.ascontiguousarray(
            f2[b].reshape(2, 128, 4096).transpose(1, 0, 2)
        ).astype(bf16)

        ys = y0[order]
        us = (ccy - np.floor(ccy))[order].astype(np.float32)
        x0 = np.floor(ccx).astype(np.int64)[order]
        vs = (ccx - np.floor(ccx))[order].astype(np.float32)

        idxc = np.zeros((128, NBLK), np.int16)
        scalc = np.zeros((128, NBLK, 16), np.float32)
        for blk in range(NBLK):
            sl = slice(blk * 128, (blk + 1) * 128)
            yb, ub, xb, vb = ys[sl], us[sl], x0[sl], vs[sl]
            tb = tbases[blk]
            p = np.arange(128)
            grp = p // 16
            sg = np.array(
                [int(yb[g * 16 : (g + 1) * 16].min()) - 4 - tb for g in range(8)]
            )
            assert (sg >= 0).all() and (sg <= rows[blk] - 10).all()
            idxc[:, blk] = (sg[grp] + (p % 16)).astype(np.int16)
            o = (yb - 4 - tb) - sg[grp]          # within-group offset
            assert (o >= 0).all() and (o <= yts[blk] - 2).all()
            # y taps: weight (1-u) at tap o, u at tap o+1
            for t in range(yts[blk]):
                scalc[:, blk, t] = np.where(o == t, 1.0 - ub, 0.0) + np.where(
                    o + 1 == t, ub, 0.0
                )
            # x taps: weight (1-v) at tap s, v at tap s+1 (s = x0 & 1)
            s = (xb & 1).astype(np.int64)
            for t in range(XT):
                scalc[:, blk, 4 + t] = np.where(s == t, 1.0 - vb, 0.0) + np.where(
                    s + 1 == t, vb, 0.0
                )
            # ladder: shift by 2*(x0 >> 1) bf16; binary bits of (x0 >> 1)
            hsh = xb >> 1
            for kst, shv in enumerate(XSH):
                scalc[:, blk, 8 + kst] = ((hsh // shv) % 2).astype(np.float32)
        in_maps.append(
            {
                "f1": f1c,
                "f2": f2c,
                "idx": np.ascontiguousarray(idxc),
                "scal": np.ascontiguousarray(scalc),
            }
        )
    return in_maps, params, [co[5] for co in cores]


def assemble(outs, orders):
    """8x [1024, 81] (sorted order; 81 = ytap*9+xtap) -> [2, 81, 64, 64].

    Reference quirk (RAFT): output tap index k = k1*9+k2 where k1 is the X
    offset and k2 the Y offset, so the x-tap axis goes in front.
    """
    full = np.zeros((NCORES, NPC, 81), np.float32)
    for i, (o, orderv) in enumerate(zip(outs, orders)):
        full[i][orderv] = np.asarray(o, dtype=np.float32)
    o = full.reshape(2, 4096, 81).reshape(2, 64, 64, 9, 9)
    return np.ascontiguousarray(
        o.transpose(0, 4, 3, 1, 2).reshape(2, 81, 64, 64)
    ).astype(np.float32)


def kernel(**inputs):
    fmap1 = np.asarray(inputs["fmap1"], np.float32)
    fmap2 = np.asarray(inputs["fmap2"], np.float32)
    coords = np.asarray(inputs["coords"], np.float32)
    radius = int(np.asarray(inputs["radius"]))
    in_maps, params, orders = host_prep(fmap1, fmap2, coords, radius)
    nc = get_nc(params)
    res = run_bass_kernel_spmd(nc, in_maps, core_ids=list(range(NCORES)))
    return assemble([r["out"] for r in res.results], orders)
